# revision 1
# baseline (speedup 1.0000x reference)
"""CeNN front-end Trainium2 kernel.

Reference computation (per batch image u [1,H,W]):
    control = conv3x3_same(u, W_B) + 0                     # [64,H,W]
    x0 = control
    x_{k+1} = alpha*x_k + beta*(conv3x3_same(tanh(x_k), WA_eff) + control + bias)
    (WA_eff = W_A with diagonal center taps clamped >= 1), 16 steps.

Distribution: 8 cores = (batch b in 0..3) x (H half). Each core owns a
272-row slab (256 valid + 16 halo rows toward the other half). Zero
communication: halo contamination advances one row per step and after 16
steps exactly the 16 halo rows are dirty.

Per-core kernel: channel-major layout [64ch -> partitions, rows, 514 cols
(W+2 zero pad)]. Rows are split into two 64-partition blocks (A on
partitions 0:64, B on 64:128) so elementwise work runs 128 wide.

conv3x3 = 9 accumulating matmuls (K=64 cin, M=64 cout, N=512) at per-tap
free offsets, in bf16 (tanh output + beta-scaled weights).  Two more
identity taps add C'' = beta*(control+bias) stored as a bf16 hi+lo
residual pair (fp32-accurate).  Four PE quadrants (tile_position) process
four rows concurrently.  The state update is one fused DVE op per row:
    x' = (x * alpha) + psum        (scalar_tensor_tensor, fp32 exact)

Time is blocked T=2 steps per pass over DRAM ping-pong buffers with
redundant-halo strips (40-row strips, 34 valid).  Pass 0 computes
control from u with a K=10 fp32 im2col matmul (9 shifted u copies + ones
row for bias).
"""

import math

import numpy as np
import ml_dtypes

import concourse.bacc as bacc
import concourse.tile as tile
from concourse import mybir
from concourse.bass_utils import run_bass_kernel_spmd

F32 = mybir.dt.float32
BF16 = mybir.dt.bfloat16
AF = mybir.ActivationFunctionType
ALU = mybir.AluOpType

FULL_CFG = dict(SLAB=272, HS=32, T=2, NSTEPS=16, RC0=17)


def _derive(cfg):
    d = dict(cfg)
    d["R"] = d["HS"] + 2 * d["T"] + 2          # strip tile rows
    assert d["R"] % 2 == 0
    d["RH"] = d["R"] // 2                      # rows per partition block
    # strips may be ragged: last strip covers the remainder
    strips = []
    o0 = 0
    while o0 < d["SLAB"]:
        hs = min(d["HS"], d["SLAB"] - o0)
        assert hs % 2 == 0
        strips.append((o0, hs))
        o0 += hs
    d["STRIPS"] = strips
    d["NSTRIP"] = len(strips)
    assert d["SLAB"] % d["RC0"] == 0
    d["NCHUNK0"] = d["SLAB"] // d["RC0"]
    assert d["NSTEPS"] % d["T"] == 0
    d["NPASS"] = d["NSTEPS"] // d["T"]
    d.setdefault("DBG_P0_OUT", d["NPASS"] == 0)
    d["UROWS"] = d["SLAB"] + 2
    return d


def build(cfg):
    """Build the per-core Bass program. Returns compiled nc."""
    g = _derive(cfg)
    SLAB, HS, T, RC0 = g["SLAB"], g["HS"], g["T"], g["RC0"]
    R, RH, NSTRIP, NCHUNK0, NPASS, UROWS = (
        g["R"], g["RH"], g["NSTRIP"], g["NCHUNK0"], g["NPASS"], g["UROWS"])
    WP = 514
    W = 512

    nc = bacc.Bacc("TRN2", target_bir_lowering=False, debug=False,
                   num_devices=8)

    u_in = nc.dram_tensor("u_in", [UROWS, W], F32, kind="ExternalInput")
    wa_in = nc.dram_tensor("wa_in", [64, 11, 64], BF16, kind="ExternalInput")
    wb_in = nc.dram_tensor("wb_in", [10, 64], F32, kind="ExternalInput")
    nbias_in = nc.dram_tensor("nbias_in", [64, 1], F32, kind="ExternalInput")
    alpha_in = nc.dram_tensor("alpha_in", [1, 1], F32, kind="ExternalInput")
    x_out = nc.dram_tensor("x_out", [64, SLAB, W], F32, kind="ExternalOutput")

    Xd = [nc.dram_tensor(f"Xd{i}", [64, SLAB, WP], F32, kind="Internal")
          for i in range(2)]
    Chi_d = nc.dram_tensor("Chi", [64, SLAB, WP], BF16, kind="Internal")
    Clo_d = nc.dram_tensor("Clo", [64, SLAB, WP], BF16, kind="Internal")

    with tile.TileContext(nc) as tc:
        with tc.tile_pool(name="singles", bufs=1) as singles:
            wa_t = singles.tile([128, 11, 64], BF16)
            nc.sync.dma_start(out=wa_t[0:64], in_=wa_in[:, :, :])
            nc.sync.dma_start(out=wa_t[64:128], in_=wa_in[:, :, :])
            wb_t = singles.tile([10, 64], F32)
            nc.sync.dma_start(out=wb_t, in_=wb_in[:, :])
            nbias_t = singles.tile([64, 1], F32)
            nc.sync.dma_start(out=nbias_t, in_=nbias_in[:, :])
            alpha_t = singles.tile([128, 1], F32)
            nc.sync.dma_start(out=alpha_t, in_=alpha_in[:, :].to_broadcast((128, 1)))
            beta_t = singles.tile([128, 1], F32)
            nc.vector.tensor_scalar(out=beta_t, in0=alpha_t, scalar1=-1.0,
                                    scalar2=1.0, op0=ALU.mult, op1=ALU.add)

            # ---------------- pass 0: control -> x0, C_hi, C_lo -------------
            with tc.tile_pool(name="p0u", bufs=2) as p0u, \
                 tc.tile_pool(name="p0ps", bufs=4, space="PSUM") as p0ps, \
                 tc.tile_pool(name="p0st", bufs=2) as p0st:
                for chk in range(NCHUNK0):
                    c0 = RC0 * chk
                    u9 = p0u.tile([10, RC0, W], F32)
                    nc.vector.memset(u9, 0.0)
                    nc.vector.memset(u9[0:1, :, :], 1.0)
                    for t9 in range(9):
                        kh, kw = divmod(t9, 3)
                        # u9[1+t9, t, c] = u_slab[c0+t+kh-1, c+kw-1]
                        c_lo = max(0, 1 - kw)
                        c_hi = min(W, W + 1 - kw)
                        nc.sync.dma_start(
                            out=u9[t9 + 1:t9 + 2, 0:RC0, c_lo:c_hi],
                            in_=u_in[c0 + kh:c0 + kh + RC0,
                                     c_lo + kw - 1:c_hi + kw - 1],
                        )
                    xst = p0st.tile([64, RC0, WP], F32, tag="xst")
                    chst = p0st.tile([64, RC0, WP], BF16, tag="chst")
                    clst = p0st.tile([64, RC0, WP], BF16, tag="clst")
                    for st in (xst, chst, clst):
                        nc.vector.memset(st[:, :, 0:1], 0.0)
                        nc.vector.memset(st[:, :, 513:514], 0.0)
                    for t in range(RC0):
                        pc = p0ps.tile([64, 512], F32)
                        nc.tensor.matmul(pc, wb_t[0:10, :], u9[0:10, t, :],
                                         start=True, stop=True)
                        # x0 = psum - bias
                        nc.scalar.activation(out=xst[:, t, 1:513], in_=pc,
                                             func=AF.Identity,
                                             bias=nbias_t[0:64], scale=1.0)
                        # C_hi = bf16(beta * psum)
                        nc.scalar.activation(out=chst[:, t, 1:513], in_=pc,
                                             func=AF.Copy,
                                             scale=beta_t[0:64])
                        # C_lo = bf16(beta*psum - C_hi)
                        nc.vector.scalar_tensor_tensor(
                            out=clst[:, t, 1:513], in0=pc,
                            scalar=beta_t[0:64], in1=chst[:, t, 1:513],
                            op0=ALU.mult, op1=ALU.subtract)
                    nc.sync.dma_start(out=Xd[0][:, c0:c0 + RC0, :], in_=xst)
                    if g["DBG_P0_OUT"]:
                        nc.sync.dma_start(out=x_out[:, c0:c0 + RC0, :],
                                          in_=xst[:, :, 1:513])
                    nc.sync.dma_start(out=Chi_d[:, c0:c0 + RC0, :], in_=chst)
                    nc.sync.dma_start(out=Clo_d[:, c0:c0 + RC0, :], in_=clst)

            # ---------------- passes 1..NPASS: T steps each ------------------
            # SBUF strip layout: contiguous row blocks. Tile rows [0, RHs) on
            # partitions 0:64 (block A), rows [RHs, Rs) on 64:128 (block B).
            # All of an interior row's taps source its own block, so each
            # PSUM accumulation group uses a single tile_position row-group
            # (mixed row-groups in one group crash the device). The two seam
            # rows (RHs-1, RHs) put their <=3 cross-block taps into a separate
            # single-source PSUM group, folded in with one extra DVE add.
            # Taps are emitted round-robin across the 4 in-flight rows so the
            # in-order PE FIFO interleaves all 4 quadrant streams.
            with tc.tile_pool(name="xs", bufs=2) as xpool, \
                 tc.tile_pool(name="chs", bufs=2) as chpool, \
                 tc.tile_pool(name="cls", bufs=2) as clpool, \
                 tc.tile_pool(name="th", bufs=g["RH"] + 4) as thpool, \
                 tc.tile_pool(name="ps", bufs=3, space="PSUM") as pspool, \
                 tc.tile_pool(name="pf", bufs=2, space="PSUM") as pfpool:
                for p in range(1, NPASS + 1):
                    src_d = Xd[(p - 1) % 2]
                    dst_d = Xd[p % 2]
                    last = (p == NPASS)
                    for (o0, hs) in g["STRIPS"]:
                        Rs = hs + 2 * T + 2
                        RHs = Rs // 2
                        base = o0 - (T + 1)          # slab row of tile row 0
                        sv_lo = max(0, -base)
                        sv_hi = min(Rs, SLAB - base)
                        xs = xpool.tile([128, RHs, WP], F32, tag="xs")
                        ch = chpool.tile([128, RHs, WP], BF16, tag="ch")
                        cl = clpool.tile([128, RHs, WP], BF16, tag="cl")
                        for blk in range(2):
                            lo, hi = blk * RHs, (blk + 1) * RHs
                            pr = slice(blk * 64, blk * 64 + 64)
                            ld_lo, ld_hi = max(lo, sv_lo), min(hi, sv_hi)
                            if ld_lo > lo:
                                nc.vector.memset(xs[pr, 0:ld_lo - lo, :], 0.0)
                            if ld_hi < hi:
                                nc.vector.memset(xs[pr, ld_hi - lo:RHs, :], 0.0)
                            nc.sync.dma_start(
                                out=xs[pr, ld_lo - lo:ld_hi - lo, :],
                                in_=src_d[:, base + ld_lo:base + ld_hi, :])
                            nc.sync.dma_start(
                                out=ch[pr, ld_lo - lo:ld_hi - lo, :],
                                in_=Chi_d[:, base + ld_lo:base + ld_hi, :])
                            nc.sync.dma_start(
                                out=cl[pr, ld_lo - lo:ld_hi - lo, :],
                                in_=Clo_d[:, base + ld_lo:base + ld_hi, :])
                        for k in range(1, T + 1):
                            up_lo = max(k, sv_lo)
                            up_hi = min(Rs - k, sv_hi)
                            th = []
                            for j in range(RHs):
                                tt = thpool.tile([128, WP], BF16)
                                nc.scalar.activation(out=tt, in_=xs[:, j, :],
                                                     func=AF.Tanh)
                                th.append(tt)

                            def row_taps(trow, ph, ps_tile, pf_tile):
                                """Build this row's matmul arg-list (main
                                group, then foreign group). Returns
                                (list of matmul kwargs, used_foreign)."""
                                dblk, dj = divmod(trow, RHs)
                                dp = slice(dblk * 64, dblk * 64 + 64)
                                out_ps = ps_tile[ph * 64:ph * 64 + 64, :]
                                main, foreign = [], []
                                for t9 in range(9):
                                    kh, kw = divmod(t9, 3)
                                    srow = trow + kh - 1
                                    sblk, sj = divmod(srow, RHs)
                                    (main if sblk == dblk else foreign).append(
                                        (t9, sblk, sj, kw))
                                ops = []
                                for i, (t9, sblk, sj, kw) in enumerate(main):
                                    ops.append(dict(
                                        out=out_ps, lhsT=wa_t[dp, t9, :],
                                        rhs=th[sj][dp, kw:kw + 512],
                                        start=(i == 0), stop=False,
                                        tile_position=(dblk * 64, ph * 64)))
                                ops.append(dict(
                                    out=out_ps, lhsT=wa_t[dp, 9, :],
                                    rhs=ch[dp, dj, 1:513],
                                    start=False, stop=False,
                                    tile_position=(dblk * 64, ph * 64)))
                                ops.append(dict(
                                    out=out_ps, lhsT=wa_t[dp, 10, :],
                                    rhs=cl[dp, dj, 1:513],
                                    start=False, stop=True,
                                    tile_position=(dblk * 64, ph * 64)))
                                if foreign:
                                    sblk = foreign[0][1]
                                    sp = slice(sblk * 64, sblk * 64 + 64)
                                    out_pf = pf_tile[ph * 64:ph * 64 + 64, :]
                                    for i, (t9, _, sj, kw) in enumerate(foreign):
                                        ops.append(dict(
                                            out=out_pf, lhsT=wa_t[sp, t9, :],
                                            rhs=th[sj][sp, kw:kw + 512],
                                            start=(i == 0),
                                            stop=(i == len(foreign) - 1),
                                            tile_position=(sblk * 64,
                                                           ph * 64)))
                                return ops, bool(foreign)

                            def upd(prow, prange, in1):
                                nc.vector.scalar_tensor_tensor(
                                    out=xs[prange, prow, 1:513],
                                    in0=xs[prange, prow, 1:513],
                                    scalar=alpha_t[prange], in1=in1,
                                    op0=ALU.mult, op1=ALU.add)

                            def fadd(prow, prange, in1):
                                nc.vector.scalar_tensor_tensor(
                                    out=xs[prange, prow, 1:513],
                                    in0=xs[prange, prow, 1:513],
                                    scalar=1.0, in1=in1,
                                    op0=ALU.bypass, op1=ALU.add)

                            def act(trow):
                                return up_lo <= trow < up_hi

                            for j0 in range(0, RHs, 2):
                                j1 = j0 + 1
                                a0, b0 = act(j0), act(RHs + j0)
                                if j1 < RHs:
                                    a1, b1 = act(j1), act(RHs + j1)
                                else:
                                    a1 = b1 = False
                                P0 = P1 = PF = None
                                need_pf = ((a0 and j0 == RHs - 1)
                                           or (a1 and j1 == RHs - 1)
                                           or (b0 and j0 == 0))
                                if need_pf:
                                    PF = pfpool.tile([128, 512], F32)
                                if a0 or b0:
                                    P0 = pspool.tile([128, 512], F32, tag="P0")
                                if a1 or b1:
                                    P1 = pspool.tile([128, 512], F32, tag="P1")
                                seqs = []
                                frows = []
                                if a0:
                                    s, f = row_taps(j0, 0, P0, PF)
                                    seqs.append(s)
                                    if f:
                                        frows.append((j0, slice(0, 64), 0))
                                if b0:
                                    s, f = row_taps(RHs + j0, 1, P0, PF)
                                    seqs.append(s)
                                    if f:
                                        frows.append((j0, slice(64, 128), 1))
                                if a1:
                                    s, f = row_taps(j1, 1, P1, PF)
                                    seqs.append(s)
                                    if f:
                                        frows.append((j1, slice(0, 64), 1))
                                if b1:
                                    s, f = row_taps(RHs + j1, 0, P1, PF)
                                    seqs.append(s)
                                    if f:
                                        frows.append((j1, slice(64, 128), 0))
                                # round-robin across rows -> 4 quadrant streams
                                nmax = max((len(s) for s in seqs), default=0)
                                for t in range(nmax):
                                    for s in seqs:
                                        if t < len(s):
                                            nc.tensor.matmul(
                                                s[t]["out"], s[t]["lhsT"],
                                                s[t]["rhs"],
                                                start=s[t]["start"],
                                                stop=s[t]["stop"],
                                                skip_group_check=True,
                                                tile_position=s[t][
                                                    "tile_position"])
                                if a0 and b0:
                                    upd(j0, slice(0, 128), P0)
                                else:
                                    if a0:
                                        upd(j0, slice(0, 64), P0[0:64, :])
                                    if b0:
                                        upd(j0, slice(64, 128), P0[64:128, :])
                                if a1:
                                    upd(j1, slice(0, 64), P1[64:128, :])
                                if b1:
                                    upd(j1, slice(64, 128), P1[0:64, :])
                                for (pj, xsl, phh) in frows:
                                    fadd(pj, xsl,
                                         PF[phh * 64:phh * 64 + 64, :])
                        # store valid rows (tile rows [T+1, Rs-T-1))
                        st_lo, st_hi = T + 1, Rs - (T + 1)
                        for blk in range(2):
                            lo, hi = blk * RHs, (blk + 1) * RHs
                            pr = slice(blk * 64, blk * 64 + 64)
                            s_lo, s_hi = max(lo, st_lo), min(hi, st_hi)
                            if s_lo >= s_hi:
                                continue
                            if last:
                                nc.sync.dma_start(
                                    out=x_out[:, base + s_lo:base + s_hi, :],
                                    in_=xs[pr, s_lo - lo:s_hi - lo, 1:513])
                            else:
                                nc.sync.dma_start(
                                    out=dst_d[:, base + s_lo:base + s_hi, :],
                                    in_=xs[pr, s_lo - lo:s_hi - lo, :])

    nc.compile()
    return nc


def host_prep(u, W_B, W_A, bias, alpha_logit, cfg):
    """Build per-core input maps. Only valid for the full-size problem."""
    g = _derive(cfg)
    SLAB, UROWS = g["SLAB"], g["UROWS"]
    B = u.shape[0]
    H = u.shape[2]
    Wc = 512

    alpha = np.float32(1.0 / (1.0 + np.exp(-np.float64(alpha_logit))))
    beta = np.float32(1.0) - alpha

    WAe = np.array(W_A, dtype=np.float32).copy()
    idx = np.arange(64)
    WAe[idx, idx, 1, 1] = np.maximum(WAe[idx, idx, 1, 1], np.float32(1.0))

    wa_taps = np.zeros((64, 11, 64), dtype=np.float32)
    for t9 in range(9):
        kh, kw = divmod(t9, 3)
        wa_taps[:, t9, :] = (beta * WAe[:, :, kh, kw]).T   # [cin, cout]
    eye = np.eye(64, dtype=np.float32)
    wa_taps[:, 9, :] = eye
    wa_taps[:, 10, :] = eye
    wa_taps = wa_taps.astype(ml_dtypes.bfloat16)

    bias_vec = np.array(bias, dtype=np.float32).reshape(64)
    wb10 = np.zeros((10, 64), dtype=np.float32)
    wb10[0, :] = bias_vec
    for t9 in range(9):
        kh, kw = divmod(t9, 3)
        wb10[t9 + 1, :] = W_B[:, 0, kh, kw]
    nbias = (-bias_vec).reshape(64, 1).astype(np.float32)
    alpha_arr = np.full((1, 1), alpha, dtype=np.float32)

    in_maps = []
    for core in range(8):
        b, h = divmod(core, 2)
        img = np.asarray(u[b, 0], dtype=np.float32)        # [H, 512]
        u_slab = np.zeros((UROWS, Wc), dtype=np.float32)
        if h == 0:
            # slab rows [-1, SLAB+1) = image rows [-1, SLAB+1)
            u_slab[1:UROWS] = img[0:SLAB + 1]
        else:
            off = H - SLAB                                  # 240
            # slab row s = image row s + off; u_in[j] = image j-1+off
            u_slab[0:UROWS - 1] = img[off - 1:H]
        in_maps.append({
            "u_in": u_slab,
            "wa_in": wa_taps,
            "wb_in": wb10,
            "nbias_in": nbias,
            "alpha_in": alpha_arr,
        })
    return in_maps


_NC_CACHE = {}


def _get_nc(cfg_key=None):
    if "nc" not in _NC_CACHE:
        _NC_CACHE["nc"] = build(FULL_CFG)
    return _NC_CACHE["nc"]


def kernel(u, W_B, W_A, bias, alpha_logit, _trace=False):
    u = np.asarray(u, dtype=np.float32)
    B, _, H, Wc = u.shape
    nc = _get_nc()
    in_maps = host_prep(u, W_B, W_A, bias, alpha_logit, FULL_CFG)
    res = run_bass_kernel_spmd(nc, in_maps, core_ids=list(range(8)),
                               trace=_trace)
    SLAB = FULL_CFG["SLAB"]
    VALID = H // 2                                          # 256
    out = np.zeros((B, 64, H, Wc), dtype=np.float32)
    for core in range(8):
        b, h = divmod(core, 2)
        xo = res.results[core]["x_out"]                     # [64, SLAB, 512]
        if h == 0:
            out[b, :, 0:VALID, :] = xo[:, 0:VALID, :]
        else:
            out[b, :, VALID:H, :] = xo[:, SLAB - VALID:SLAB, :]
    kernel._last_results = res
    return out



# revision 5
# speedup vs baseline: 1.0061x; 1.0061x over previous
"""CeNN front-end Trainium2 kernel — time-skewed (wavefront) schedule.

Reference computation (per batch image u [1,H,W]):
    control = conv3x3_same(u, W_B)                         # [64,H,W]
    x0 = control
    x_{k+1} = alpha*x_k + beta*(conv3x3_same(tanh(x_k), WA_eff) + control
                                + bias)      (WA_eff diag center >= 1), 16x.

Distribution: 8 cores = (batch b 0..3) x (H half).  Each core owns a
272-row slab (256 valid + 16 halo rows); halo contamination advances one
row per step so zero inter-core communication is needed.

Schedule: T=4 steps per DRAM pass (4 passes).  Within a pass the slab is
processed as NT=8 sequential 34-row tiles, TIME-SKEWED: tile n at step k
updates rows (b_{n-1}-k, b_n-k].  The 2-row x_k boundary needed by tile
n+1 at step k+1 is carried tile-to-tile in SBUF (carry_1 into main slots
{2,3}; carry_2/3 into dedicated D slots), so NO halo rows are ever
recomputed or reloaded — each pass does exactly SLAB rows of conv per
step and loads/stores each state row once.

State x and control C = beta*(control+bias) are bf16 throughout
(validated rel err ~4e-3 vs 2e-2 budget).  conv3x3 = 9 accumulating
quadrant matmuls (K=64, M=64, N=512) + 1 identity tap adding C, four PE
quadrants running 4 row-streams concurrently.  Update per row is one
fused DVE op: x' = x*alpha + psum.

SBUF tile layout per 34-row tile (parity pi alternates per tile so
carries stay in their partition half): window slot s in [0,39]:
s<20 -> (parts of pi, j=s); s in [20,38] -> (other half, j=s-20).
Dedicated top-half rows: j20=CS0 (x0 boundary row), j21..24 = D2/D3
carry pairs.
"""

import numpy as np
import ml_dtypes

import concourse.bacc as bacc
import concourse.tile as tile
from concourse import mybir
from concourse.bass_utils import run_bass_kernel_spmd

F32 = mybir.dt.float32
BF16 = mybir.dt.bfloat16
AF = mybir.ActivationFunctionType
ALU = mybir.AluOpType

SLAB, W, WP = 272, 512, 514
T, HS = 4, 34
NT = SLAB // HS
RH = 20
NJ_TOP = 25            # j: 0..19 main, 20 CS0, 21/22 D2, 23/24 D3
CS0 = 40
DSLOT = {2: (41, 42), 3: (43, 44)}
NPASS = 16 // T
RC0 = 17
NCHUNK0 = SLAB // RC0
UROWS = SLAB + 2


def _half(s, pi):
    if s >= 40:
        return pi
    return pi if s < RH else 1 - pi


def _j(s):
    if s >= 40:
        return s - 20
    return s if s < RH else s - RH


def _P(h):
    return slice(64 * h, 64 * h + 64)


def _ssrc(n, k, d):
    """Slot holding x_{k-1}[b0+d] when tile n runs step k (d >= -k)."""
    if n == 1:
        return d + T
    if k == 1:
        return CS0 if d == -1 else d + T
    if k == 2:
        return d + T
    if d == -k:
        return DSLOT[k - 1][0]
    if d == -(k - 1):
        return DSLOT[k - 1][1]
    return d + T


def build():
    nc = bacc.Bacc("TRN2", target_bir_lowering=False, debug=False,
                   num_devices=8)

    u_in = nc.dram_tensor("u_in", [UROWS, W], F32, kind="ExternalInput")
    wa_in = nc.dram_tensor("wa_in", [64, 10, 64], BF16, kind="ExternalInput")
    wb_in = nc.dram_tensor("wb_in", [10, 64], F32, kind="ExternalInput")
    nbias_in = nc.dram_tensor("nbias_in", [64, 1], F32, kind="ExternalInput")
    alpha_in = nc.dram_tensor("alpha_in", [1, 1], F32, kind="ExternalInput")
    x_out = nc.dram_tensor("x_out", [64, SLAB, W], BF16,
                           kind="ExternalOutput")

    Xd = [nc.dram_tensor(f"Xd{i}", [64, SLAB, WP], BF16, kind="Internal")
          for i in range(2)]
    Chi_d = nc.dram_tensor("Chi", [64, SLAB, WP], BF16, kind="Internal")

    with tile.TileContext(nc) as tc:
        with tc.tile_pool(name="singles", bufs=1) as singles:
            wa_t = singles.tile([128, 10, 64], BF16)
            nc.sync.dma_start(out=wa_t[0:64], in_=wa_in[:, :, :])
            nc.sync.dma_start(out=wa_t[64:128], in_=wa_in[:, :, :])
            wb_t = singles.tile([10, 64], F32)
            nc.sync.dma_start(out=wb_t, in_=wb_in[:, :])
            nbias_t = singles.tile([64, 1], F32)
            nc.sync.dma_start(out=nbias_t, in_=nbias_in[:, :])
            alpha_t = singles.tile([128, 1], F32)
            nc.sync.dma_start(out=alpha_t,
                              in_=alpha_in[:, :].to_broadcast((128, 1)))
            beta_t = singles.tile([128, 1], F32)
            nc.vector.tensor_scalar(out=beta_t, in0=alpha_t, scalar1=-1.0,
                                    scalar2=1.0, op0=ALU.mult, op1=ALU.add)
            zrow = singles.tile([64, SLAB, 1], BF16)
            nc.vector.memset(zrow, 0.0)
            # Xd1 edge columns must be zero (Xd0/Chi get theirs from pass0)
            nc.sync.dma_start(out=Xd[1][:, :, 0:1], in_=zrow)
            nc.sync.dma_start(out=Xd[1][:, :, 513:514], in_=zrow)

            # ---------------- pass 0: x0 = control, C = beta*(ctrl+bias) ----
            with tc.tile_pool(name="p0u", bufs=2) as p0u, \
                 tc.tile_pool(name="p0ps", bufs=4, space="PSUM") as p0ps, \
                 tc.tile_pool(name="p0st", bufs=2) as p0st:
                for chk in range(NCHUNK0):
                    c0 = RC0 * chk
                    u9 = p0u.tile([10, RC0, W], F32)
                    nc.vector.memset(u9, 0.0)
                    nc.vector.memset(u9[0:1, :, :], 1.0)
                    for t9 in range(9):
                        kh, kw = divmod(t9, 3)
                        c_lo = max(0, 1 - kw)
                        c_hi = min(W, W + 1 - kw)
                        nc.sync.dma_start(
                            out=u9[t9 + 1:t9 + 2, 0:RC0, c_lo:c_hi],
                            in_=u_in[c0 + kh:c0 + kh + RC0,
                                     c_lo + kw - 1:c_hi + kw - 1],
                        )
                    xst = p0st.tile([64, RC0, WP], BF16, tag="xst")
                    chst = p0st.tile([64, RC0, WP], BF16, tag="chst")
                    for st in (xst, chst):
                        nc.vector.memset(st[:, :, 0:1], 0.0)
                        nc.vector.memset(st[:, :, 513:514], 0.0)
                    for t in range(RC0):
                        pc = p0ps.tile([64, 512], F32)
                        nc.tensor.matmul(pc, wb_t[0:10, :], u9[0:10, t, :],
                                         start=True, stop=True)
                        # x0 = psum - bias (wb row 0 added bias)
                        nc.scalar.activation(out=xst[:, t, 1:513], in_=pc,
                                             func=AF.Identity,
                                             bias=nbias_t[0:64], scale=1.0)
                        # C = beta * psum
                        nc.vector.tensor_scalar(
                            out=chst[:, t, 1:513], in0=pc,
                            scalar1=beta_t[0:64], scalar2=None, op0=ALU.mult)
                    nc.sync.dma_start(out=Xd[0][:, c0:c0 + RC0, :], in_=xst)
                    nc.sync.dma_start(out=Chi_d[:, c0:c0 + RC0, :], in_=chst)

            # ---------------- passes 1..NPASS ------------------------------
            with tc.tile_pool(name="xs", bufs=2) as xpool, \
                 tc.tile_pool(name="chs", bufs=2) as chpool, \
                 tc.tile_pool(name="th", bufs=26) as thpool, \
                 tc.tile_pool(name="ps", bufs=3, space="PSUM") as pspool, \
                 tc.tile_pool(name="pf", bufs=2, space="PSUM") as pfpool:
                for p in range(1, NPASS + 1):
                    src_d = Xd[(p - 1) % 2]
                    dst_d = Xd[p % 2]
                    last = (p == NPASS)
                    xs_cur = xpool.tile([128, NJ_TOP, WP], BF16, tag="xs")
                    for n in range(1, NT + 1):
                        pi = (n - 1) % 2
                        b0 = (n - 1) * HS
                        xs = xs_cur
                        if n < NT:
                            xs_nxt = xpool.tile([128, NJ_TOP, WP], BF16,
                                                tag="xs", name="xs_nxt")
                        else:
                            xs_nxt = None
                        ch = chpool.tile([128, RH, WP], BF16, tag="ch")
                        _emit_tile(nc, n, pi, b0, xs, xs_nxt, ch, src_d,
                                   dst_d, Chi_d, x_out, wa_t, alpha_t,
                                   thpool, pspool, pfpool, last)
                        xs_cur = xs_nxt

    nc.compile()
    return nc


def _emit_tile(nc, n, pi, b0, xs, xs_nxt, ch, src_d, dst_d, Chi_d, x_out,
               wa_t, alpha_t, thpool, pspool, pfpool, last):
    TOP, BOT = _P(pi), _P(1 - pi)

    # ---------------- loads ----------------
    if n == 1:
        nc.vector.memset(xs[TOP, 2:4, :], 0.0)     # rows -2,-1
        nc.vector.memset(xs[TOP, 20:21, :], 0.0)   # CS0 (row -1 at k=1)
    else:
        nc.sync.dma_start(out=xs[TOP, 20:21, :],
                          in_=src_d[:, b0 - 1:b0, :])
        # slot 1 (top j1) is written only at k=T cols 1:513 -> zero edges
        nc.vector.memset(xs[TOP, 1:2, :], 0.0)
    # main slots 4..19 = rows b0..b0+15 ; 20..38 = rows b0+16..b0+34
    nc.sync.dma_start(out=xs[TOP, 4:20, :], in_=src_d[:, b0:b0 + 16, :])
    if n == NT:
        nc.sync.dma_start(out=xs[BOT, 0:18, :],
                          in_=src_d[:, b0 + 16:b0 + 34, :])
        nc.vector.memset(xs[BOT, 18:19, :], 0.0)   # row 272 = 0
    else:
        nc.sync.dma_start(out=xs[BOT, 0:19, :],
                          in_=src_d[:, b0 + 16:b0 + 35, :])
    # ch: slots 1..37 (updated rows); n=1: 4..37
    if n == 1:
        nc.sync.dma_start(out=ch[TOP, 4:20, :], in_=Chi_d[:, 0:16, :])
    else:
        nc.sync.dma_start(out=ch[TOP, 1:20, :],
                          in_=Chi_d[:, b0 - 3:b0 + 16, :])
    nc.sync.dma_start(out=ch[BOT, 0:18, :],
                      in_=Chi_d[:, b0 + 16:b0 + 34, :])

    # ---------------- steps ----------------
    for k in range(1, T + 1):
        r_lo = 0 if n == 1 else b0 - k + 1
        r_hi = SLAB - 1 if n == NT else b0 + HS - k
        s_lo = r_lo - b0 + T
        s_hi = r_hi - b0 + T

        def ssrc(d):
            return _ssrc(n, k, d)

        # ---- tanh of all source slots ----
        need = sorted({ssrc(r - b0) for r in range(r_lo - 1, r_hi + 2)})
        th = {}
        need_tb = {}
        for s in need:
            h, j = _half(s, pi), _j(s)
            need_tb.setdefault(j, set()).add(h)
        for j, hs_ in sorted(need_tb.items()):
            tt = thpool.tile([128, WP], BF16)
            th[j] = tt
            if hs_ == {0, 1}:
                nc.scalar.activation(out=tt, in_=xs[:, j, :], func=AF.Tanh)
            else:
                (h,) = hs_
                nc.scalar.activation(out=tt[_P(h)], in_=xs[_P(h), j, :],
                                     func=AF.Tanh)

        # ---- group rows ----
        # pairable j: top slot j and bot slot j+20 both updated, no remap
        remap_j = None
        if n > 1 and k >= 3:
            remap_j = T - k + 1        # in0 comes from D slot
        pair_js = [j for j in range(s_lo, s_hi - RH + 1) if j != remap_j]
        top_singles = [j for j in range(max(s_lo, s_hi - RH + 1), RH)]
        if remap_j is not None and remap_j >= s_lo:
            top_singles.insert(0, remap_j)
        bot_singles = [j for j in range(0, s_hi - RH + 1)
                       if j not in pair_js]

        groups = []
        i = 0
        while i + 1 < len(pair_js):
            groups.append(("g4", pair_js[i], pair_js[i + 1]))
            i += 2
        if i < len(pair_js):
            groups.append(("g2p", pair_js[i], None))
        ti = bi = 0
        while ti < len(top_singles) or bi < len(bot_singles):
            jt = top_singles[ti] if ti < len(top_singles) else None
            jb = bot_singles[bi] if bi < len(bot_singles) else None
            groups.append(("g2m", jt, jb))
            ti += 1
            bi += 1

        def row_taps(s_out, ph, ps_tile, pf_tile):
            d = s_out - T
            dhalf = _half(s_out, pi)
            dj = _j(s_out)
            out_ps = ps_tile[ph * 64:ph * 64 + 64, :]
            main, foreign = [], []
            for t9 in range(9):
                kh, kw = divmod(t9, 3)
                ss = ssrc(d + kh - 1)
                shalf, sj = _half(ss, pi), _j(ss)
                ent = (t9, shalf, sj, kw)
                (main if shalf == dhalf else foreign).append(ent)
            ops = []
            for idx, (t9, shalf, sj, kw) in enumerate(main):
                ops.append(dict(
                    out=out_ps, lhsT=wa_t[_P(shalf), t9, :],
                    rhs=th[sj][_P(shalf), kw:kw + 512],
                    start=(idx == 0), stop=False,
                    tile_position=(shalf * 64, ph * 64)))
            ops.append(dict(
                out=out_ps, lhsT=wa_t[_P(dhalf), 9, :],
                rhs=ch[_P(dhalf), dj, 1:513],
                start=False, stop=True,
                tile_position=(dhalf * 64, ph * 64)))
            if foreign:
                fhalf = foreign[0][1]
                out_pf = pf_tile[ph * 64:ph * 64 + 64, :]
                for idx, (t9, shalf, sj, kw) in enumerate(foreign):
                    ops.append(dict(
                        out=out_pf, lhsT=wa_t[_P(shalf), t9, :],
                        rhs=th[sj][_P(shalf), kw:kw + 512],
                        start=(idx == 0), stop=(idx == len(foreign) - 1),
                        tile_position=(shalf * 64, ph * 64)))
            return ops, bool(foreign)

        def upd_full(j, ps_tile):
            nc.vector.scalar_tensor_tensor(
                out=xs[:, j, 1:513], in0=xs[:, j, 1:513],
                scalar=alpha_t, in1=ps_tile[:, :],
                op0=ALU.mult, op1=ALU.add)

        def upd_half(s_out, ph, ps_tile):
            dhalf = _half(s_out, pi)
            dj = _j(s_out)
            d = s_out - T
            sin = ssrc(d)
            jin = _j(sin)
            nc.vector.scalar_tensor_tensor(
                out=xs[_P(dhalf), dj, 1:513],
                in0=xs[_P(dhalf), jin, 1:513],
                scalar=alpha_t[_P(dhalf)],
                in1=ps_tile[_P(ph), :],
                op0=ALU.mult, op1=ALU.add)

        def fadd(s_out, ph, pf_tile):
            dhalf = _half(s_out, pi)
            dj = _j(s_out)
            nc.vector.scalar_tensor_tensor(
                out=xs[_P(dhalf), dj, 1:513],
                in0=xs[_P(dhalf), dj, 1:513],
                scalar=1.0, in1=pf_tile[_P(ph), :],
                op0=ALU.bypass, op1=ALU.add)

        for g in groups:
            kind = g[0]
            seqs = []
            folds = []   # (s_out, ph, pf)
            upds = []    # callables
            PF = None

            def need_foreign(s_out):
                d = s_out - T
                dhalf = _half(s_out, pi)
                for kh in (0, 2):
                    if _half(ssrc(d + kh - 1), pi) != dhalf:
                        return True
                return False

            if kind == "g4":
                _, ja, jb = g
                P0 = pspool.tile([128, 512], F32, tag="P0")
                P1 = pspool.tile([128, 512], F32, tag="P1")
                plan = [(ja, pi, P0), (ja + RH, 1 - pi, P0),
                        (jb, 1 - pi, P1), (jb + RH, pi, P1)]
            elif kind == "g2p":
                _, ja, _unused = g
                P0 = pspool.tile([128, 512], F32, tag="P0")
                plan = [(ja, pi, P0), (ja + RH, 1 - pi, P0)]
            else:  # g2m
                _, jt, jb = g
                P0 = pspool.tile([128, 512], F32, tag="P0")
                plan = []
                if jt is not None:
                    plan.append((jt, pi, P0))
                if jb is not None:
                    plan.append((jb + RH, 1 - pi, P0))
            if any(need_foreign(s) for (s, _, _) in plan):
                PF = pfpool.tile([128, 512], F32)
            for (s_out, ph, Pt) in plan:
                ops, f = row_taps(s_out, ph, Pt, PF)
                seqs.append(ops)
                if f:
                    folds.append((s_out, ph))
            nmax = max((len(s) for s in seqs), default=0)
            for t in range(nmax):
                for s in seqs:
                    if t < len(s):
                        nc.tensor.matmul(
                            s[t]["out"], s[t]["lhsT"], s[t]["rhs"],
                            start=s[t]["start"], stop=s[t]["stop"],
                            skip_group_check=True,
                            tile_position=s[t]["tile_position"])
            # updates
            if kind == "g4":
                _, ja, jb = g
                upd_full(ja, P0)
                upd_half(jb, 1 - pi, P1)          # top jb, psum other half
                upd_half(jb + RH, pi, P1)         # bot jb
            elif kind == "g2p":
                _, ja, _unused = g
                upd_full(ja, P0)
            else:
                _, jt, jb = g
                if jt is not None:
                    upd_half(jt, pi, P0)
                if jb is not None:
                    upd_half(jb + RH, 1 - pi, P0)
            for (s_out, ph) in folds:
                fadd(s_out, ph, PF)

        # ---- carry x_k boundary rows to tile n+1 ----
        if k < T and xs_nxt is not None:
            s_src = HS + T - k - 1          # rows b_n-k-1, b_n-k (bot half)
            j_src = s_src - RH
            if k == 1:
                dj0 = 2                      # -> main top slots {2,3}
            else:
                dj0 = _j(DSLOT[k][0])        # -> D_k pair
            # tile n+1 top half == this tile's bot half (parity alternates)
            nc.vector.tensor_copy(out=xs_nxt[BOT, dj0:dj0 + 2, :],
                                  in_=xs[BOT, j_src:j_src + 2, :])

    # ---------------- store x_T ----------------
    s_lo_st = 4 if n == 1 else 1
    s_hi_st = (SLAB - 1 - b0 + T) if n == NT else HS
    # top slots s_lo_st..19 ; bot slots 20..s_hi_st
    r_top0 = b0 - T + s_lo_st
    n_top = RH - s_lo_st
    r_bot0 = b0 - T + RH
    n_bot = s_hi_st - RH + 1
    if last:
        nc.sync.dma_start(out=x_out[:, r_top0:r_top0 + n_top, :],
                          in_=xs[_P(pi), s_lo_st:RH, 1:513])
        nc.sync.dma_start(out=x_out[:, r_bot0:r_bot0 + n_bot, :],
                          in_=xs[_P(1 - pi), 0:n_bot, 1:513])
    else:
        nc.sync.dma_start(out=dst_d[:, r_top0:r_top0 + n_top, 1:513],
                          in_=xs[_P(pi), s_lo_st:RH, 1:513])
        nc.sync.dma_start(out=dst_d[:, r_bot0:r_bot0 + n_bot, 1:513],
                          in_=xs[_P(1 - pi), 0:n_bot, 1:513])


def host_prep(u, W_B, W_A, bias, alpha_logit):
    alpha = np.float32(1.0 / (1.0 + np.exp(-np.float64(alpha_logit))))
    beta = np.float32(1.0) - alpha

    WAe = np.array(W_A, dtype=np.float32).copy()
    idx = np.arange(64)
    WAe[idx, idx, 1, 1] = np.maximum(WAe[idx, idx, 1, 1], np.float32(1.0))

    wa_taps = np.zeros((64, 10, 64), dtype=np.float32)
    for t9 in range(9):
        kh, kw = divmod(t9, 3)
        wa_taps[:, t9, :] = (beta * WAe[:, :, kh, kw]).T   # [cin, cout]
    wa_taps[:, 9, :] = np.eye(64, dtype=np.float32)
    wa_taps = wa_taps.astype(ml_dtypes.bfloat16)

    bias_vec = np.array(bias, dtype=np.float32).reshape(64)
    wb10 = np.zeros((10, 64), dtype=np.float32)
    wb10[0, :] = bias_vec
    for t9 in range(9):
        kh, kw = divmod(t9, 3)
        wb10[t9 + 1, :] = W_B[:, 0, kh, kw]
    nbias = (-bias_vec).reshape(64, 1).astype(np.float32)
    alpha_arr = np.full((1, 1), alpha, dtype=np.float32)

    H = u.shape[2]
    in_maps = []
    for core in range(8):
        b, h = divmod(core, 2)
        img = np.asarray(u[b, 0], dtype=np.float32)        # [H, 512]
        u_slab = np.zeros((UROWS, W), dtype=np.float32)
        if h == 0:
            u_slab[1:UROWS] = img[0:SLAB + 1]
        else:
            off = H - SLAB
            u_slab[0:UROWS - 1] = img[off - 1:H]
        in_maps.append({
            "u_in": u_slab,
            "wa_in": wa_taps,
            "wb_in": wb10,
            "nbias_in": nbias,
            "alpha_in": alpha_arr,
        })
    return in_maps


_NC_CACHE = {}


def _get_nc():
    if "nc" not in _NC_CACHE:
        _NC_CACHE["nc"] = build()
    return _NC_CACHE["nc"]


def kernel(u, W_B, W_A, bias, alpha_logit, _trace=False):
    u = np.asarray(u, dtype=np.float32)
    B, _, H, Wc = u.shape
    nc = _get_nc()
    in_maps = host_prep(u, W_B, W_A, bias, alpha_logit)
    res = run_bass_kernel_spmd(nc, in_maps, core_ids=list(range(8)),
                               trace=_trace)
    VALID = H // 2
    out = np.zeros((B, 64, H, Wc), dtype=np.float32)
    for core in range(8):
        b, h = divmod(core, 2)
        xo = np.asarray(res.results[core]["x_out"]).astype(np.float32)
        if h == 0:
            out[b, :, 0:VALID, :] = xo[:, 0:VALID, :]
        else:
            out[b, :, VALID:H, :] = xo[:, SLAB - VALID:SLAB, :]
    kernel._last_results = res
    return out


# revision 7
# speedup vs baseline: 1.0123x; 1.0062x over previous
"""CeNN front-end Trainium2 kernel — time-skewed (wavefront) schedule.

Reference computation (per batch image u [1,H,W]):
    control = conv3x3_same(u, W_B)                         # [64,H,W]
    x0 = control
    x_{k+1} = alpha*x_k + beta*(conv3x3_same(tanh(x_k), WA_eff) + control
                                + bias)      (WA_eff diag center >= 1), 16x.

Distribution: 8 cores = (batch b 0..3) x (H half).  Each core owns a
272-row slab (256 valid + 16 halo rows); halo contamination advances one
row per step so zero inter-core communication is needed.

Schedule: T=4 steps per DRAM pass (4 passes).  Within a pass the slab is
processed as NT=8 sequential 34-row tiles, TIME-SKEWED: tile n at step k
updates rows (b_{n-1}-k, b_n-k].  The 2-row x_k boundary needed by tile
n+1 at step k+1 is carried tile-to-tile in SBUF (carry_1 into main slots
{2,3}; carry_2/3 into dedicated D slots), so NO halo rows are ever
recomputed or reloaded — each pass does exactly SLAB rows of conv per
step and loads/stores each state row once.

Pass 0 (control via K=10 im2col matmul from u) is emitted interleaved
with pass-1 tiles so its ACT/DVE/DMA work fills pass-1's engine slack.

Next-step tanh ops are emitted immediately after the group that
completes their source rows, so the PE never waits a full step-boundary
for ACT; carry-fed slots (CS0/D/carry_1 dests) are tanh'd at tile start
from held buffers.

State x and control C = beta*(control+bias) are bf16 throughout
(validated rel err ~4.5e-3 vs 2e-2 budget).  conv3x3 = 9 accumulating
quadrant matmuls (K=64, M=64, N=512) + 1 identity tap adding C, four PE
quadrants running 4 row-streams concurrently.  Update per row is one
fused DVE op: x' = x*alpha + psum.
"""

import numpy as np
import ml_dtypes

import concourse.bacc as bacc
import concourse.tile as tile
from concourse import mybir
from concourse.bass_utils import run_bass_kernel_spmd

F32 = mybir.dt.float32
BF16 = mybir.dt.bfloat16
AF = mybir.ActivationFunctionType
ALU = mybir.AluOpType

SLAB, W, WP = 272, 512, 514
T, HS = 4, 34
NT = SLAB // HS
RH = 20
NJ_TOP = 25            # j: 0..19 main, 20 CS0, 21/22 D2, 23/24 D3
CS0 = 40
DSLOT = {2: (41, 42), 3: (43, 44)}
NPASS = 16 // T
RC0 = 8
NCHUNK0 = SLAB // RC0
UROWS = SLAB + 2


def _half(s, pi):
    if s >= 40:
        return pi
    return pi if s < RH else 1 - pi


def _j(s):
    if s >= 40:
        return s - 20
    return s if s < RH else s - RH


def _P(h):
    return slice(64 * h, 64 * h + 64)


def _ssrc(n, k, d):
    """Slot holding x_{k-1}[b0+d] when tile n runs step k (d >= -k)."""
    if n == 1:
        return d + T
    if k == 1:
        return CS0 if d == -1 else d + T
    if k == 2:
        return d + T
    if d == -k:
        return DSLOT[k - 1][0]
    if d == -(k - 1):
        return DSLOT[k - 1][1]
    return d + T


def _tile_geom(n, k):
    b0 = (n - 1) * HS
    r_lo = 0 if n == 1 else b0 - k + 1
    r_hi = SLAB - 1 if n == NT else b0 + HS - k
    return b0, r_lo, r_hi, r_lo - b0 + T, r_hi - b0 + T


def _plan_step(n, k):
    """Group plan for tile n step k.  Returns (groups, written) where
    groups is a list of (kind, ja, jb) and written maps slot -> group
    index of its writer."""
    b0, r_lo, r_hi, s_lo, s_hi = _tile_geom(n, k)
    remap_j = None
    if n > 1 and k >= 3:
        remap_j = T - k + 1
    pair_js = [j for j in range(s_lo, s_hi - RH + 1) if j != remap_j]
    top_singles = [j for j in range(max(s_lo, s_hi - RH + 1), RH)]
    if remap_j is not None and remap_j >= s_lo:
        top_singles.insert(0, remap_j)
    bot_singles = [j for j in range(0, s_hi - RH + 1) if j not in pair_js]

    groups = []
    i = 0
    while i + 1 < len(pair_js):
        groups.append(("g4", pair_js[i], pair_js[i + 1]))
        i += 2
    if i < len(pair_js):
        groups.append(("g2p", pair_js[i], None))
    ti = bi = 0
    while ti < len(top_singles) or bi < len(bot_singles):
        jt = top_singles[ti] if ti < len(top_singles) else None
        jb = bot_singles[bi] if bi < len(bot_singles) else None
        groups.append(("g2m", jt, jb))
        ti += 1
        bi += 1

    written = {}
    for gi, (kind, ja, jb) in enumerate(groups):
        if kind == "g4":
            for s in (ja, ja + RH, jb, jb + RH):
                written[s] = gi
        elif kind == "g2p":
            written[ja] = gi
            written[ja + RH] = gi
        else:
            if ja is not None:
                written[ja] = gi
            if jb is not None:
                written[jb + RH] = gi
    return groups, written


def _need_tb(n, k, pi):
    """Tanh coverage for step k: {j: set(halves)} over source slots."""
    b0, r_lo, r_hi, _, _ = _tile_geom(n, k)
    need = {}
    for r in range(r_lo - 1, r_hi + 2):
        s = _ssrc(n, k, r - b0)
        need.setdefault(_j(s), set()).add(_half(s, pi))
    return need


def build():
    nc = bacc.Bacc("TRN2", target_bir_lowering=False, debug=False,
                   num_devices=8)

    u_in = nc.dram_tensor("u_in", [UROWS, W], BF16, kind="ExternalInput")
    wa_in = nc.dram_tensor("wa_in", [64, 10, 64], BF16, kind="ExternalInput")
    wb_in = nc.dram_tensor("wb_in", [10, 64], BF16, kind="ExternalInput")
    nbias_in = nc.dram_tensor("nbias_in", [64, 1], F32, kind="ExternalInput")
    alpha_in = nc.dram_tensor("alpha_in", [1, 1], F32, kind="ExternalInput")
    x_out = nc.dram_tensor("x_out", [64, SLAB, W], BF16,
                           kind="ExternalOutput")

    Xd = [nc.dram_tensor(f"Xd{i}", [64, SLAB, WP], BF16, kind="Internal")
          for i in range(2)]
    Chi_d = nc.dram_tensor("Chi", [64, SLAB, WP], BF16, kind="Internal")

    with tile.TileContext(nc) as tc:
        with tc.tile_pool(name="singles", bufs=1) as singles:
            wa_t = singles.tile([128, 10, 64], BF16)
            nc.sync.dma_start(out=wa_t[0:64], in_=wa_in[:, :, :])
            nc.sync.dma_start(out=wa_t[64:128], in_=wa_in[:, :, :])
            wb_t = singles.tile([10, 64], BF16)
            nc.sync.dma_start(out=wb_t, in_=wb_in[:, :])
            nbias_t = singles.tile([64, 1], F32)
            nc.sync.dma_start(out=nbias_t, in_=nbias_in[:, :])
            alpha_t = singles.tile([128, 1], F32)
            nc.sync.dma_start(out=alpha_t,
                              in_=alpha_in[:, :].to_broadcast((128, 1)))
            beta_t = singles.tile([128, 1], F32)
            nc.vector.tensor_scalar(out=beta_t, in0=alpha_t, scalar1=-1.0,
                                    scalar2=1.0, op0=ALU.mult, op1=ALU.add)
            zrow = singles.tile([64, SLAB, 1], BF16)
            nc.vector.memset(zrow, 0.0)
            # Xd1 edge columns must be zero (Xd0/Chi get theirs from pass0)
            nc.sync.dma_start(out=Xd[1][:, :, 0:1], in_=zrow)
            nc.sync.dma_start(out=Xd[1][:, :, 513:514], in_=zrow)

            with tc.tile_pool(name="p0u", bufs=2) as p0u, \
                 tc.tile_pool(name="p0st", bufs=2) as p0st, \
                 tc.tile_pool(name="xs", bufs=2) as xpool, \
                 tc.tile_pool(name="chs", bufs=2) as chpool, \
                 tc.tile_pool(name="th", bufs=40) as thpool, \
                 tc.tile_pool(name="ps", bufs=3, space="PSUM") as pspool:

                def emit_chunk(c):
                    c0 = RC0 * c
                    u9 = p0u.tile([10, RC0, W], BF16, name="u9")
                    nc.vector.memset(u9, 0.0)
                    nc.vector.memset(u9[0:1, :, :], 1.0)
                    for t9 in range(9):
                        kh, kw = divmod(t9, 3)
                        c_lo = max(0, 1 - kw)
                        c_hi = min(W, W + 1 - kw)
                        nc.sync.dma_start(
                            out=u9[t9 + 1:t9 + 2, 0:RC0, c_lo:c_hi],
                            in_=u_in[c0 + kh:c0 + kh + RC0,
                                     c_lo + kw - 1:c_hi + kw - 1],
                        )
                    xst = p0st.tile([64, RC0, WP], BF16, tag="xst",
                                    name="xst")
                    chst = p0st.tile([64, RC0, WP], BF16, tag="chst",
                                     name="chst")
                    for st in (xst, chst):
                        nc.vector.memset(st[:, :, 0:1], 0.0)
                        nc.vector.memset(st[:, :, 513:514], 0.0)
                    for t in range(RC0):
                        pc = pspool.tile([64, 512], F32, tag="pc", bufs=2,
                                         name="pc")
                        nc.tensor.matmul(pc, wb_t[0:10, :], u9[0:10, t, :],
                                         start=True, stop=True)
                        nc.scalar.activation(out=xst[:, t, 1:513], in_=pc,
                                             func=AF.Identity,
                                             bias=nbias_t[0:64], scale=1.0)
                        nc.vector.tensor_scalar(
                            out=chst[:, t, 1:513], in0=pc,
                            scalar1=beta_t[0:64], scalar2=None, op0=ALU.mult)
                    nc.sync.dma_start(out=Xd[0][:, c0:c0 + RC0, :], in_=xst)
                    nc.sync.dma_start(out=Chi_d[:, c0:c0 + RC0, :], in_=chst)

                chunks_done = 0
                for p in range(1, NPASS + 1):
                    src_d = Xd[(p - 1) % 2]
                    dst_d = Xd[p % 2]
                    last = (p == NPASS)
                    xs_cur = xpool.tile([128, NJ_TOP, WP], BF16, tag="xs")
                    for n in range(1, NT + 1):
                        if p == 1:
                            need_c = min(NCHUNK0,
                                         -(-(n * HS + 36) // RC0))
                            if n == NT:
                                need_c = NCHUNK0
                            while chunks_done < need_c:
                                emit_chunk(chunks_done)
                                chunks_done += 1
                        pi = (n - 1) % 2
                        xs = xs_cur
                        if n < NT:
                            xs_nxt = xpool.tile([128, NJ_TOP, WP], BF16,
                                                tag="xs", name="xs_nxt")
                        else:
                            xs_nxt = None
                        ch = chpool.tile([128, RH, WP], BF16, tag="ch")
                        _emit_tile(nc, n, pi, xs, xs_nxt, ch, src_d,
                                   dst_d, Chi_d, x_out, wa_t, alpha_t,
                                   thpool, pspool, last)
                        xs_cur = xs_nxt

    nc.compile()
    return nc


def _emit_tile(nc, n, pi, xs, xs_nxt, ch, src_d, dst_d, Chi_d, x_out,
               wa_t, alpha_t, thpool, pspool, last):
    TOP, BOT = _P(pi), _P(1 - pi)
    b0 = (n - 1) * HS

    # ---------------- loads ----------------
    if n == 1:
        nc.vector.memset(xs[TOP, 2:4, :], 0.0)     # rows -2,-1
        nc.vector.memset(xs[TOP, 20:21, :], 0.0)   # CS0 (row -1 at k=1)
    else:
        nc.sync.dma_start(out=xs[TOP, 20:21, :],
                          in_=src_d[:, b0 - 1:b0, :])
        # slot 1 (top j1) is written only at k=T cols 1:513 -> zero edges
        nc.vector.memset(xs[TOP, 1:2, :], 0.0)
    nc.sync.dma_start(out=xs[TOP, 4:20, :], in_=src_d[:, b0:b0 + 16, :])
    if n == NT:
        nc.sync.dma_start(out=xs[BOT, 0:18, :],
                          in_=src_d[:, b0 + 16:b0 + 34, :])
        nc.vector.memset(xs[BOT, 18:19, :], 0.0)   # row 272 = 0
    else:
        nc.sync.dma_start(out=xs[BOT, 0:19, :],
                          in_=src_d[:, b0 + 16:b0 + 35, :])
    if n == 1:
        nc.sync.dma_start(out=ch[TOP, 4:20, :], in_=Chi_d[:, 0:16, :])
    else:
        nc.sync.dma_start(out=ch[TOP, 1:20, :],
                          in_=Chi_d[:, b0 - 3:b0 + 16, :])
    nc.sync.dma_start(out=ch[BOT, 0:18, :],
                      in_=Chi_d[:, b0 + 16:b0 + 34, :])

    # ---------------- plan all steps ----------------
    plans = {}
    for k in range(1, T + 1):
        groups, written = _plan_step(n, k)
        need = _need_tb(n, k, pi)
        plans[k] = (groups, written, need)

    # th tiles per (k, j); gates: (k, j) -> group index in step k-1 or -1
    thd = {k: {} for k in range(1, T + 1)}
    gates = {}
    for k in range(1, T + 1):
        _, _, need = plans[k]
        w_prev = plans[k - 1][1] if k > 1 else {}
        for j, hs_ in need.items():
            g = -1
            for h in hs_:
                if j >= RH:
                    s = j + 20          # dedicated (top half by defn)
                else:
                    s = j if h == pi else j + RH
                if s in w_prev:
                    g = max(g, w_prev[s])
            gates[(k, j)] = g

    def emit_tanh(k, j):
        hs_ = plans[k][2][j]
        if j >= RH:
            # dedicated carry slots live until step k's matmuls; keep them
            # out of the main rotation so ACT never stalls on their WAR
            tt = thpool.tile([128, WP], BF16, name="tth", tag="hold",
                             bufs=12)
        else:
            tt = thpool.tile([128, WP], BF16, name="tt")
        thd[k][j] = tt
        if hs_ == {0, 1}:
            nc.scalar.activation(out=tt, in_=xs[:, j, :], func=AF.Tanh)
        else:
            (h,) = hs_
            nc.scalar.activation(out=tt[_P(h)], in_=xs[_P(h), j, :],
                                 func=AF.Tanh)

    # start-gated tanhs (carry/D/CS0/zero slots + all of step 1)
    for k in range(1, T + 1):
        for j in sorted(plans[k][2]):
            if gates[(k, j)] < 0:
                emit_tanh(k, j)

    # ---------------- steps ----------------
    for k in range(1, T + 1):
        groups, written, need = plans[k]
        th = thd[k]
        b0_, r_lo, r_hi, s_lo, s_hi = _tile_geom(n, k)

        def ssrc(d):
            return _ssrc(n, k, d)

        def row_taps(s_out, ph, ps_tile, pf_tile):
            d = s_out - T
            dhalf = _half(s_out, pi)
            dj = _j(s_out)
            out_ps = ps_tile[ph * 64:ph * 64 + 64, :]
            main, foreign = [], []
            for t9 in range(9):
                kh, kw = divmod(t9, 3)
                ss = ssrc(d + kh - 1)
                shalf, sj = _half(ss, pi), _j(ss)
                ent = (t9, shalf, sj, kw)
                (main if shalf == dhalf else foreign).append(ent)
            ops = []
            for idx, (t9, shalf, sj, kw) in enumerate(main):
                ops.append(dict(
                    out=out_ps, lhsT=wa_t[_P(shalf), t9, :],
                    rhs=th[sj][_P(shalf), kw:kw + 512],
                    start=(idx == 0), stop=False,
                    tile_position=(shalf * 64, ph * 64)))
            ops.append(dict(
                out=out_ps, lhsT=wa_t[_P(dhalf), 9, :],
                rhs=ch[_P(dhalf), dj, 1:513],
                start=False, stop=True,
                tile_position=(dhalf * 64, ph * 64)))
            if foreign:
                out_pf = pf_tile[ph * 64:ph * 64 + 64, :]
                for idx, (t9, shalf, sj, kw) in enumerate(foreign):
                    ops.append(dict(
                        out=out_pf, lhsT=wa_t[_P(shalf), t9, :],
                        rhs=th[sj][_P(shalf), kw:kw + 512],
                        start=(idx == 0), stop=(idx == len(foreign) - 1),
                        tile_position=(shalf * 64, ph * 64)))
            return ops, bool(foreign)

        def upd_full(j, ps_tile):
            nc.vector.scalar_tensor_tensor(
                out=xs[:, j, 1:513], in0=xs[:, j, 1:513],
                scalar=alpha_t, in1=ps_tile[:, :],
                op0=ALU.mult, op1=ALU.add)

        def upd_half(s_out, ph, ps_tile):
            dhalf = _half(s_out, pi)
            dj = _j(s_out)
            jin = _j(ssrc(s_out - T))
            nc.vector.scalar_tensor_tensor(
                out=xs[_P(dhalf), dj, 1:513],
                in0=xs[_P(dhalf), jin, 1:513],
                scalar=alpha_t[_P(dhalf)],
                in1=ps_tile[_P(ph), :],
                op0=ALU.mult, op1=ALU.add)

        def fadd(s_out, ph, pf_tile):
            dhalf = _half(s_out, pi)
            dj = _j(s_out)
            nc.vector.scalar_tensor_tensor(
                out=xs[_P(dhalf), dj, 1:513],
                in0=xs[_P(dhalf), dj, 1:513],
                scalar=1.0, in1=pf_tile[_P(ph), :],
                op0=ALU.bypass, op1=ALU.add)

        def need_foreign(s_out):
            d = s_out - T
            dhalf = _half(s_out, pi)
            for kh in (0, 2):
                if _half(ssrc(d + kh - 1), pi) != dhalf:
                    return True
            return False

        # carry source slots & their writer group
        carry_gi = -1
        if k < T and xs_nxt is not None:
            carry_gi = max(written[HS + T - k - 1], written[HS + T - k])

        for gi, g in enumerate(groups):
            kind = g[0]
            seqs = []
            folds = []
            PF = None
            if kind == "g4":
                _, ja, jb = g
                P0 = pspool.tile([128, 512], F32, tag="P0", name="P0")
                P1 = pspool.tile([128, 512], F32, tag="P1", bufs=2,
                                 name="P1")
                plan = [(ja, pi, P0), (ja + RH, 1 - pi, P0),
                        (jb, 1 - pi, P1), (jb + RH, pi, P1)]
            elif kind == "g2p":
                _, ja, _u = g
                P0 = pspool.tile([128, 512], F32, tag="P0", name="P0")
                P1 = None
                plan = [(ja, pi, P0), (ja + RH, 1 - pi, P0)]
            else:
                _, jt, jb = g
                P0 = pspool.tile([128, 512], F32, tag="P0", name="P0")
                P1 = None
                plan = []
                if jt is not None:
                    plan.append((jt, pi, P0))
                if jb is not None:
                    plan.append((jb + RH, 1 - pi, P0))
            if any(need_foreign(s) for (s, _, _) in plan):
                PF = pspool.tile([128, 512], F32, tag="PF", bufs=1,
                                 name="PF")
            for (s_out, ph, Pt) in plan:
                ops, f = row_taps(s_out, ph, Pt, PF)
                seqs.append(ops)
                if f:
                    folds.append((s_out, ph))
            nmax = max((len(s) for s in seqs), default=0)
            for t in range(nmax):
                for s in seqs:
                    if t < len(s):
                        nc.tensor.matmul(
                            s[t]["out"], s[t]["lhsT"], s[t]["rhs"],
                            start=s[t]["start"], stop=s[t]["stop"],
                            skip_group_check=True,
                            tile_position=s[t]["tile_position"])
            if kind == "g4":
                _, ja, jb = g
                upd_full(ja, P0)
                upd_half(jb, 1 - pi, P1)
                upd_half(jb + RH, pi, P1)
            elif kind == "g2p":
                _, ja, _u = g
                upd_full(ja, P0)
            else:
                _, jt, jb = g
                if jt is not None:
                    upd_half(jt, pi, P0)
                if jb is not None:
                    upd_half(jb + RH, 1 - pi, P0)
            for (s_out, ph) in folds:
                fadd(s_out, ph, PF)
            # carry as soon as its source rows are final
            if gi == carry_gi:
                s_src = HS + T - k - 1
                j_src = s_src - RH
                dj0 = 2 if k == 1 else _j(DSLOT[k][0])
                nc.vector.tensor_copy(out=xs_nxt[BOT, dj0:dj0 + 2, :],
                                      in_=xs[BOT, j_src:j_src + 2, :])
            # next-step tanhs gated on this group
            if k < T:
                for j in sorted(plans[k + 1][2]):
                    if gates[(k + 1, j)] == gi:
                        emit_tanh(k + 1, j)

    # ---------------- store x_T ----------------
    s_lo_st = 4 if n == 1 else 1
    s_hi_st = (SLAB - 1 - b0 + T) if n == NT else HS
    r_top0 = b0 - T + s_lo_st
    n_top = RH - s_lo_st
    r_bot0 = b0 - T + RH
    n_bot = s_hi_st - RH + 1
    if last:
        nc.sync.dma_start(out=x_out[:, r_top0:r_top0 + n_top, :],
                          in_=xs[_P(pi), s_lo_st:RH, 1:513])
        nc.sync.dma_start(out=x_out[:, r_bot0:r_bot0 + n_bot, :],
                          in_=xs[_P(1 - pi), 0:n_bot, 1:513])
    else:
        nc.sync.dma_start(out=dst_d[:, r_top0:r_top0 + n_top, 1:513],
                          in_=xs[_P(pi), s_lo_st:RH, 1:513])
        nc.sync.dma_start(out=dst_d[:, r_bot0:r_bot0 + n_bot, 1:513],
                          in_=xs[_P(1 - pi), 0:n_bot, 1:513])


def host_prep(u, W_B, W_A, bias, alpha_logit):
    alpha = np.float32(1.0 / (1.0 + np.exp(-np.float64(alpha_logit))))
    beta = np.float32(1.0) - alpha

    WAe = np.array(W_A, dtype=np.float32).copy()
    idx = np.arange(64)
    WAe[idx, idx, 1, 1] = np.maximum(WAe[idx, idx, 1, 1], np.float32(1.0))

    wa_taps = np.zeros((64, 10, 64), dtype=np.float32)
    for t9 in range(9):
        kh, kw = divmod(t9, 3)
        wa_taps[:, t9, :] = (beta * WAe[:, :, kh, kw]).T   # [cin, cout]
    wa_taps[:, 9, :] = np.eye(64, dtype=np.float32)
    wa_taps = wa_taps.astype(ml_dtypes.bfloat16)

    bias_vec = np.array(bias, dtype=np.float32).reshape(64)
    wb10 = np.zeros((10, 64), dtype=np.float32)
    wb10[0, :] = bias_vec
    for t9 in range(9):
        kh, kw = divmod(t9, 3)
        wb10[t9 + 1, :] = W_B[:, 0, kh, kw]
    wb10 = wb10.astype(ml_dtypes.bfloat16)
    nbias = (-bias_vec).reshape(64, 1).astype(np.float32)
    alpha_arr = np.full((1, 1), alpha, dtype=np.float32)

    H = u.shape[2]
    in_maps = []
    for core in range(8):
        b, h = divmod(core, 2)
        img = np.asarray(u[b, 0], dtype=np.float32)        # [H, 512]
        u_slab = np.zeros((UROWS, W), dtype=np.float32)
        if h == 0:
            u_slab[1:UROWS] = img[0:SLAB + 1]
        else:
            off = H - SLAB
            u_slab[0:UROWS - 1] = img[off - 1:H]
        in_maps.append({
            "u_in": u_slab.astype(ml_dtypes.bfloat16),
            "wa_in": wa_taps,
            "wb_in": wb10,
            "nbias_in": nbias,
            "alpha_in": alpha_arr,
        })
    return in_maps


_NC_CACHE = {}


def _get_nc():
    if "nc" not in _NC_CACHE:
        _NC_CACHE["nc"] = build()
    return _NC_CACHE["nc"]


def kernel(u, W_B, W_A, bias, alpha_logit, _trace=False):
    u = np.asarray(u, dtype=np.float32)
    B, _, H, Wc = u.shape
    nc = _get_nc()
    in_maps = host_prep(u, W_B, W_A, bias, alpha_logit)
    res = run_bass_kernel_spmd(nc, in_maps, core_ids=list(range(8)),
                               trace=_trace)
    VALID = H // 2
    out = np.zeros((B, 64, H, Wc), dtype=np.float32)
    for core in range(8):
        b, h = divmod(core, 2)
        xo = np.asarray(res.results[core]["x_out"]).astype(np.float32)
        if h == 0:
            out[b, :, 0:VALID, :] = xo[:, 0:VALID, :]
        else:
            out[b, :, VALID:H, :] = xo[:, SLAB - VALID:SLAB, :]
    kernel._last_results = res
    return out
